# revision 8
# baseline (speedup 1.0000x reference)
"""ConvShapeletFilter kernel for Trainium2 (8 NeuronCores, data-parallel).

Math: reference computes, per batch row b and shapelet k,
    corr[b,n,k] = <x_win[b,n]-mean(x_win[b,n]), s[k]-mean(s[k])>
Since (s[k]-mean(s[k])) sums to zero over l, the window-mean term drops:
    corr[b,n,k] = sum_l x[b,n+l] * s_norm[k,l]
i.e. a plain cross-correlation with the mean-removed shapelet bank.
Outputs per (b,k): top-1, mean(top-5), top-2, relu(top1-top2) over n.

Device mapping (per core, 32 of 256 batch rows), bf16 datapath:
  - s_norm^T prepared host-side, shipped bf16 as [L, K] (full bank).
  - Hankel tile per batch row: h[l, c] = x[b, l + c], l in 0..127 --
    ONE overlapping-AP DMA of [128, TPAD-G] bf16 (128 descriptors of
    ~8.4KB).  The single tile serves every window block: block at
    col j is one full-contraction matmul snt^T.T @ h[:, j:j+512].
    128-row contraction keeps the PE instruction count at 8/row.
  - Hankel DMAs alternate between the two HWDGE queues (sync, scalar)
    so ~4 DMA rings run concurrently.
  - corr spans of [K, 2048] fp32 in PSUM (2 spans x 4 banks,
    double-buffered); DVE InstMax (top-8) per span + small merge;
    finalize ops on Scalar/DVE; one PE transpose + 4 DMAs write y.
"""

import os
import sys

for _p in ("/opt/trn_rl_repo", os.path.expanduser("~/.axon_site/_ro/trn_rl_repo")):
    if os.path.isdir(_p) and _p not in sys.path:
        sys.path.insert(0, _p)

import numpy as np

B, T = 256, 4096
L = 128
K = 128
K_TOP = 5
N = T - L + 1          # 3969 sliding windows
N_CORES = 8
ROWS = B // N_CORES    # 32 batch rows per core
WBLK = 512             # windows per matmul (1 PSUM bank fp32)
SPAN = 2048            # windows per PSUM span (4 banks)
OUT_COLS = 4 * K       # p1 | p_mean | p2 | dominance
G = 128                # hankel partitions (full contraction)
TPAD = 4352            # padded x row: shifts 0..127 + cols 0..4223


def _split_excess_waits(nc, mybir, max_waits=1):
    """Walrus CoreV3 codegen rejects >1 sync-wait on several instruction
    classes (CTRL/Drain, S3_LW/Matmult, ...). Hoist excess waits onto
    same-engine NoOps placed just before the offender."""
    for fn in nc.m.functions:
        for bb in fn.blocks:
            insts = bb.instructions
            i = 0
            while i < len(insts):
                inst = insts[i]
                si = inst.sync_info
                if (si is not None and si.on_wait
                        and len(si.on_wait) > max_waits):
                    waits = list(si.on_wait)
                    si.on_wait = waits[:max_waits]
                    for cs in range(max_waits, len(waits), max_waits):
                        chunk = waits[cs:cs + max_waits]
                        d = nc.sync.nop(nofuse=True)
                        cur = nc.cur_bb.bb.instructions
                        assert cur[-1] is d.ins
                        cur.pop()
                        d.ins.engine = inst.engine
                        d.ins.sync_info = mybir.SyncInfo(on_wait=chunk, on_update=[])
                        insts.insert(i, d.ins)
                        i += 1
                i += 1


def build_program():
    import concourse.bass as bass
    import concourse.mybir as mybir
    from concourse.masks import make_identity
    from concourse.tile import TileContext

    f32 = mybir.dt.float32
    io_dt = mybir.dt.bfloat16

    nc = bass.Bass()
    x = nc.declare_dram_parameter("x", [ROWS, TPAD], io_dt, isOutput=False)
    snt_in = nc.declare_dram_parameter("snt", [L, K], io_dt, isOutput=False)
    y = nc.declare_dram_parameter("y", [ROWS, OUT_COLS], f32, isOutput=True)

    def hankel_ap(b, l0, nl):
        """AP over x: dims (l, c) -> x[b, l0 + l + c]."""
        ap = x[b:b + 1, 0:TPAD - G].copy()
        ap.ap = mybir.VecI64Pair([[1, nl], [1, TPAD - G]])
        ap.offset = b * TPAD + l0
        return ap

    with TileContext(nc) as tc:
        with (
            tc.tile_pool(name="const", bufs=1) as const_pool,
            tc.tile_pool(name="hank", bufs=3) as hank_pool,
            tc.tile_pool(name="cand", bufs=3) as cand_pool,
            tc.tile_pool(name="rtop", bufs=3) as rtop_pool,
        ):
            snt = const_pool.tile([L, K], io_dt)
            nc.sync.dma_start(out=snt[:, :], in_=snt_in[:, :])
            ident = const_pool.tile([128, 128], f32)
            make_identity(nc, ident[:, :])
            # Result accumulator R[k, m*32 + b], m in (p1, p_mean, p2, dom).
            R = const_pool.tile([K, 128], f32)

            spans = [(0, SPAN), (SPAN, N - SPAN)]   # (n0, n_windows)

            with tc.tile_pool(name="psum", bufs=2, space="PSUM") as psum_pool:
                for b in range(ROWS):
                    # Split each hankel load across both HWDGE queues so the
                    # two descriptor streams run in parallel.
                    h = hank_pool.tile([G, TPAD - G], io_dt, tag="hank")
                    nc.sync.dma_start(out=h[0:G // 2, :],
                                      in_=hankel_ap(b, 0, G // 2))
                    nc.scalar.dma_start(out=h[G // 2:G, :],
                                        in_=hankel_ap(b, G // 2, G // 2))

                    cand = cand_pool.tile([K, 16], f32)
                    for hi, (n0, nw) in enumerate(spans):
                        ps = psum_pool.tile([K, SPAN], f32, tag="psum")
                        for j in range(0, SPAN, WBLK):
                            c = n0 + j
                            nc.tensor.matmul(
                                ps[:, j:j + WBLK],
                                snt[:, :], h[:, c:c + WBLK],
                                start=True, stop=True)
                        nc.vector.max(out=cand[:, 8 * hi:8 * (hi + 1)],
                                      in_=ps[:, :nw])

                    rt = rtop_pool.tile([K, 8], f32)
                    nc.vector.max(out=rt[:, :], in_=cand[:, :])
                    # p1, p_mean, p2, dominance -> R cols b, 32+b, 64+b, 96+b.
                    nc.scalar.copy(R[:, b:b + 1], rt[:, 0:1])
                    pm_scratch = rtop_pool.tile([K, K_TOP], f32, tag="pmscr")
                    nc.scalar.activation(pm_scratch[:, :], rt[:, 0:K_TOP],
                                         mybir.ActivationFunctionType.Copy,
                                         scale=1.0 / K_TOP,
                                         accum_out=R[:, 32 + b:33 + b])
                    nc.scalar.copy(R[:, 64 + b:65 + b], rt[:, 1:2])
                    # dominance = relu(p1 - p2) = Relu(-p2 + bias=p1),
                    # on the scalar engine: stays off the DVE bottleneck.
                    nc.scalar.activation(R[:, 96 + b:97 + b], rt[:, 1:2],
                                         mybir.ActivationFunctionType.Relu,
                                         scale=-1.0, bias=rt[:, 0:1])

            # Four per-metric transposes R[:, m*32:(m+1)*32] -> [32, 128]
            # packed as tr[b, m*128+k], so y writes as two DMAs whose
            # descriptors are full 2KB y rows.
            with tc.tile_pool(name="tpsum", bufs=1, space="PSUM") as tpsum_pool:
                tr_ps = tpsum_pool.tile([ROWS, OUT_COLS], f32)
                for m in range(4):
                    nc.tensor.transpose(tr_ps[:, m * K:(m + 1) * K],
                                        R[:, m * ROWS:(m + 1) * ROWS],
                                        ident[:, :])
                tr = const_pool.tile([ROWS, OUT_COLS], f32)
                nc.scalar.copy(tr[:, :], tr_ps[:, :])
                half = ROWS // 2
                nc.sync.dma_start(out=y[0:half, :], in_=tr[0:half, :])
                nc.scalar.dma_start(out=y[half:ROWS, :], in_=tr[half:ROWS, :])

    _split_excess_waits(nc, mybir)
    return nc


_CACHED = {}


def _get_program():
    if "p" not in _CACHED:
        _CACHED["p"] = build_program()
    return _CACHED["p"]


def _prep_inputs(x, shapelets):
    import ml_dtypes
    x = np.ascontiguousarray(x, dtype=np.float32)
    s = np.asarray(shapelets, dtype=np.float32)
    snt = np.ascontiguousarray((s - s.mean(axis=1, keepdims=True)).T)
    x = np.pad(x, ((0, 0), (0, TPAD - T)))
    x = x.astype(ml_dtypes.bfloat16)
    snt = np.ascontiguousarray(snt).astype(ml_dtypes.bfloat16)
    return x, snt


def run_sharded(x, shapelets, trace=False, **kw):
    from concourse.bass_utils import run_bass_kernel_spmd

    nc = _get_program()
    xp, snt = _prep_inputs(x, shapelets)
    in_maps = [
        {"x": xp[c * ROWS:(c + 1) * ROWS], "snt": snt}
        for c in range(N_CORES)
    ]
    res = run_bass_kernel_spmd(nc, in_maps, list(range(N_CORES)), trace=trace, **kw)
    out = np.concatenate([res.results[c]["y"] for c in range(N_CORES)], axis=0)
    return out, res


def kernel(x, shapelets):
    out, _ = run_sharded(x, shapelets)
    return out


# revision 11
# speedup vs baseline: 1.0972x; 1.0972x over previous
"""ConvShapeletFilter kernel for Trainium2 (8 NeuronCores, data-parallel).

Math: reference computes, per batch row b and shapelet k,
    corr[b,n,k] = <x_win[b,n]-mean(x_win[b,n]), s[k]-mean(s[k])>
Since (s[k]-mean(s[k])) sums to zero over l, the window-mean term drops:
    corr[b,n,k] = sum_l x[b,n+l] * s_norm[k,l]
i.e. a plain cross-correlation with the mean-removed shapelet bank.
Outputs per (b,k): top-1, mean(top-5), top-2, relu(top1-top2) over n.

Device mapping (per core, 32 of 256 batch rows), bf16 datapath:
  - s_norm^T prepared host-side, shipped bf16 as [L, K] (full bank).
  - Hankel tile per batch row: h[l, c] = x[b, l + c], l in 0..127 --
    ONE overlapping-AP DMA of [128, TPAD-G] bf16 (128 descriptors of
    ~8.4KB).  The single tile serves every window block: block at
    col j is one full-contraction matmul snt^T.T @ h[:, j:j+512].
    128-row contraction keeps the PE instruction count at 8/row.
  - Hankel DMAs alternate between the two HWDGE queues (sync, scalar)
    so ~4 DMA rings run concurrently.
  - corr spans of [K, 2048] fp32 in PSUM (2 spans x 4 banks,
    double-buffered); DVE InstMax (top-8) per span + small merge;
    finalize ops on Scalar/DVE; one PE transpose + 4 DMAs write y.
"""

import os
import sys

for _p in ("/opt/trn_rl_repo", os.path.expanduser("~/.axon_site/_ro/trn_rl_repo")):
    if os.path.isdir(_p) and _p not in sys.path:
        sys.path.insert(0, _p)

import numpy as np

B, T = 256, 4096
L = 128
K = 128
K_TOP = 5
N = T - L + 1          # 3969 sliding windows
N_CORES = 8
ROWS = B // N_CORES    # 32 batch rows per core
WBLK = 512             # windows per matmul (1 PSUM bank fp32)
SPAN = 2048            # windows per PSUM span (4 banks)
OUT_COLS = 4 * K       # p1 | p_mean | p2 | dominance
G = 128                # hankel partitions (full contraction)
TPAD = 4352            # padded x row: shifts 0..127 + cols 0..4223


def _split_excess_waits(nc, mybir, max_waits=1):
    """Walrus CoreV3 codegen rejects >1 sync-wait on several instruction
    classes (CTRL/Drain, S3_LW/Matmult, ...). Hoist excess waits onto
    same-engine NoOps placed just before the offender."""
    for fn in nc.m.functions:
        for bb in fn.blocks:
            insts = bb.instructions
            i = 0
            while i < len(insts):
                inst = insts[i]
                si = inst.sync_info
                if (si is not None and si.on_wait
                        and len(si.on_wait) > max_waits):
                    waits = list(si.on_wait)
                    si.on_wait = waits[:max_waits]
                    for cs in range(max_waits, len(waits), max_waits):
                        chunk = waits[cs:cs + max_waits]
                        d = nc.sync.nop(nofuse=True)
                        cur = nc.cur_bb.bb.instructions
                        assert cur[-1] is d.ins
                        cur.pop()
                        d.ins.engine = inst.engine
                        d.ins.sync_info = mybir.SyncInfo(on_wait=chunk, on_update=[])
                        insts.insert(i, d.ins)
                        i += 1
                i += 1


def build_program():
    import concourse.bass as bass
    import concourse.mybir as mybir
    from concourse.masks import make_identity
    from concourse.tile import TileContext

    f32 = mybir.dt.float32
    io_dt = mybir.dt.bfloat16

    nc = bass.Bass()
    x = nc.declare_dram_parameter("x", [ROWS, TPAD], io_dt, isOutput=False)
    snt_in = nc.declare_dram_parameter("snt", [L, K], io_dt, isOutput=False)
    y = nc.declare_dram_parameter("y", [ROWS, OUT_COLS], f32, isOutput=True)

    def hankel_ap(b, l0, nl):
        """AP over x: dims (l, c) -> x[b, l0 + l + c]."""
        ap = x[b:b + 1, 0:TPAD - G].copy()
        ap.ap = mybir.VecI64Pair([[1, nl], [1, TPAD - G]])
        ap.offset = b * TPAD + l0
        return ap

    with TileContext(nc) as tc:
        with (
            tc.tile_pool(name="const", bufs=1) as const_pool,
            tc.tile_pool(name="hank", bufs=3) as hank_pool,
            tc.tile_pool(name="cand", bufs=3) as cand_pool,
            tc.tile_pool(name="rtop", bufs=3) as rtop_pool,
        ):
            # Split the snt load across both queues: its 128 small
            # descriptors otherwise delay row 0's hankel fan-out by ~8us.
            snt = const_pool.tile([L, K], io_dt)
            nc.sync.dma_start(out=snt[0:L // 2, :], in_=snt_in[0:L // 2, :])
            nc.scalar.dma_start(out=snt[L // 2:L, :], in_=snt_in[L // 2:L, :])
            ident = const_pool.tile([128, 128], f32)
            make_identity(nc, ident[:, :])
            # Result accumulator R[k, m*32 + b], m in (p1, p_mean, p2, dom).
            R = const_pool.tile([K, 128], f32)

            spans = [(0, SPAN), (SPAN, N - SPAN)]   # (n0, n_windows)

            with tc.tile_pool(name="psum", bufs=2, space="PSUM") as psum_pool:
                for b in range(ROWS):
                    # Steady state: whole-row hankel DMAs alternate between
                    # the two HWDGE queues (both stripe over all 16 rings;
                    # interleaving two streams per row was measured slower).
                    # First rows: split across queues to halve the pipeline
                    # fill before the first matmul.
                    h = hank_pool.tile([G, TPAD - G], io_dt, tag="hank")
                    if b < 3:
                        nc.sync.dma_start(out=h[0:G // 2, :],
                                          in_=hankel_ap(b, 0, G // 2))
                        nc.scalar.dma_start(out=h[G // 2:G, :],
                                            in_=hankel_ap(b, G // 2, G // 2))
                    else:
                        eng = (nc.sync, nc.scalar)[b % 2]
                        eng.dma_start(out=h[:, :], in_=hankel_ap(b, 0, G))

                    cand = cand_pool.tile([K, 16], f32)
                    for hi, (n0, nw) in enumerate(spans):
                        ps = psum_pool.tile([K, SPAN], f32, tag="psum")
                        for j in range(0, SPAN, WBLK):
                            c = n0 + j
                            nc.tensor.matmul(
                                ps[:, j:j + WBLK],
                                snt[:, :], h[:, c:c + WBLK],
                                start=True, stop=True)
                        nc.vector.max(out=cand[:, 8 * hi:8 * (hi + 1)],
                                      in_=ps[:, :nw])

                    rt = rtop_pool.tile([K, 8], f32)
                    nc.vector.max(out=rt[:, :], in_=cand[:, :])
                    # p1, p_mean, p2, dominance -> R cols b, 32+b, 64+b, 96+b.
                    nc.scalar.copy(R[:, b:b + 1], rt[:, 0:1])
                    pm_scratch = rtop_pool.tile([K, K_TOP], f32, tag="pmscr")
                    nc.scalar.activation(pm_scratch[:, :], rt[:, 0:K_TOP],
                                         mybir.ActivationFunctionType.Copy,
                                         scale=1.0 / K_TOP,
                                         accum_out=R[:, 32 + b:33 + b])
                    nc.scalar.copy(R[:, 64 + b:65 + b], rt[:, 1:2])
                    # dominance = relu(p1 - p2) = Relu(-p2 + bias=p1),
                    # on the scalar engine: stays off the DVE bottleneck.
                    nc.scalar.activation(R[:, 96 + b:97 + b], rt[:, 1:2],
                                         mybir.ActivationFunctionType.Relu,
                                         scale=-1.0, bias=rt[:, 0:1])

                    # Write y in two half-batches so only the second half's
                    # (short) chain sits in the drain. Per-metric transposes
                    # R[:, m*32+r0 : m*32+r0+16] -> [16, 128] packed as
                    # tr[b, m*128+k]; descriptors are then full 2KB y rows.
                    if b in (15, ROWS - 1):
                        r0 = 0 if b == 15 else 16
                        ps_tr = psum_pool.tile([K, SPAN], f32, tag="psum")
                        for m in range(4):
                            nc.tensor.transpose(
                                ps_tr[0:16, m * K:(m + 1) * K],
                                R[:, m * ROWS + r0:m * ROWS + r0 + 16],
                                ident[:, :])
                        trh = const_pool.tile([16, OUT_COLS], f32,
                                              tag=f"trh{r0}")
                        nc.scalar.copy(trh[:, :], ps_tr[0:16, 0:OUT_COLS])
                        eng = (nc.sync, nc.scalar)[b == 15]
                        eng.dma_start(out=y[r0:r0 + 16, :], in_=trh[:, :])

    _split_excess_waits(nc, mybir)
    return nc


_CACHED = {}


def _get_program():
    if "p" not in _CACHED:
        _CACHED["p"] = build_program()
    return _CACHED["p"]


def _prep_inputs(x, shapelets):
    import ml_dtypes
    x = np.ascontiguousarray(x, dtype=np.float32)
    s = np.asarray(shapelets, dtype=np.float32)
    snt = np.ascontiguousarray((s - s.mean(axis=1, keepdims=True)).T)
    x = np.pad(x, ((0, 0), (0, TPAD - T)))
    x = x.astype(ml_dtypes.bfloat16)
    snt = np.ascontiguousarray(snt).astype(ml_dtypes.bfloat16)
    return x, snt


def run_sharded(x, shapelets, trace=False, **kw):
    from concourse.bass_utils import run_bass_kernel_spmd

    nc = _get_program()
    xp, snt = _prep_inputs(x, shapelets)
    in_maps = [
        {"x": xp[c * ROWS:(c + 1) * ROWS], "snt": snt}
        for c in range(N_CORES)
    ]
    res = run_bass_kernel_spmd(nc, in_maps, list(range(N_CORES)), trace=trace, **kw)
    out = np.concatenate([res.results[c]["y"] for c in range(N_CORES)], axis=0)
    return out, res


def kernel(x, shapelets):
    out, _ = run_sharded(x, shapelets)
    return out


# revision 14
# speedup vs baseline: 1.1048x; 1.0070x over previous
"""ConvShapeletFilter kernel for Trainium2 (8 NeuronCores, data-parallel).

Math: reference computes, per batch row b and shapelet k,
    corr[b,n,k] = <x_win[b,n]-mean(x_win[b,n]), s[k]-mean(s[k])>
Since (s[k]-mean(s[k])) sums to zero over l, the window-mean term drops:
    corr[b,n,k] = sum_l x[b,n+l] * s_norm[k,l]
i.e. a plain cross-correlation with the mean-removed shapelet bank.
Outputs per (b,k): top-1, mean(top-5), top-2, relu(top1-top2) over n.

Device mapping (per core, 32 of 256 batch rows), bf16 datapath:
  - s_norm^T prepared host-side, shipped bf16 as [L, K] (full bank).
  - Hankel tile per batch row: h[l, c] = x[b, l + c], l in 0..127 --
    ONE overlapping-AP DMA of [128, TPAD-G] bf16 (128 descriptors of
    ~8.4KB).  The single tile serves every window block: block at
    col j is one full-contraction matmul snt^T.T @ h[:, j:j+512].
    128-row contraction keeps the PE instruction count at 8/row.
  - Hankel DMAs alternate between the two HWDGE queues (sync, scalar)
    so ~4 DMA rings run concurrently.
  - corr spans of [K, 2048] fp32 in PSUM (2 spans x 4 banks,
    double-buffered); DVE InstMax (top-8) per span + small merge;
    finalize ops on Scalar/DVE; one PE transpose + 4 DMAs write y.
"""

import os
import sys

for _p in ("/opt/trn_rl_repo", os.path.expanduser("~/.axon_site/_ro/trn_rl_repo")):
    if os.path.isdir(_p) and _p not in sys.path:
        sys.path.insert(0, _p)

import numpy as np

B, T = 256, 4096
L = 128
K = 128
K_TOP = 5
N = T - L + 1          # 3969 sliding windows
N_CORES = 8
ROWS = B // N_CORES    # 32 batch rows per core
WBLK = 512             # windows per matmul (1 PSUM bank fp32)
SPAN = 2048            # windows per PSUM span (4 banks)
OUT_COLS = 4 * K       # p1 | p_mean | p2 | dominance
G = 128                # hankel partitions (full contraction)
TPAD = 4352            # padded x row: shifts 0..127 + cols 0..4223


def _split_excess_waits(nc, mybir, max_waits=1):
    """Walrus CoreV3 codegen rejects >1 sync-wait on several instruction
    classes (CTRL/Drain, S3_LW/Matmult, ...). Hoist excess waits onto
    same-engine NoOps placed just before the offender."""
    for fn in nc.m.functions:
        for bb in fn.blocks:
            insts = bb.instructions
            i = 0
            while i < len(insts):
                inst = insts[i]
                si = inst.sync_info
                if (si is not None and si.on_wait
                        and len(si.on_wait) > max_waits):
                    waits = list(si.on_wait)
                    si.on_wait = waits[:max_waits]
                    for cs in range(max_waits, len(waits), max_waits):
                        chunk = waits[cs:cs + max_waits]
                        d = nc.sync.nop(nofuse=True)
                        cur = nc.cur_bb.bb.instructions
                        assert cur[-1] is d.ins
                        cur.pop()
                        d.ins.engine = inst.engine
                        d.ins.sync_info = mybir.SyncInfo(on_wait=chunk, on_update=[])
                        insts.insert(i, d.ins)
                        i += 1
                i += 1


def build_program():
    import concourse.bass as bass
    import concourse.mybir as mybir
    from concourse.masks import make_identity
    from concourse.tile import TileContext

    f32 = mybir.dt.float32
    io_dt = mybir.dt.bfloat16

    nc = bass.Bass()
    x = nc.declare_dram_parameter("x", [ROWS, TPAD], io_dt, isOutput=False)
    snt_in = nc.declare_dram_parameter("snt", [L, K], io_dt, isOutput=False)
    y = nc.declare_dram_parameter("y", [ROWS, OUT_COLS], f32, isOutput=True)

    def hankel_ap(b, l0, nl):
        """AP over x: dims (l, c) -> x[b, l0 + l + c]."""
        ap = x[b:b + 1, 0:TPAD - G].copy()
        ap.ap = mybir.VecI64Pair([[1, nl], [1, TPAD - G]])
        ap.offset = b * TPAD + l0
        return ap

    with TileContext(nc) as tc:
        with (
            tc.tile_pool(name="const", bufs=1) as const_pool,
            tc.tile_pool(name="hank", bufs=3) as hank_pool,
            tc.tile_pool(name="cand", bufs=3) as cand_pool,
            tc.tile_pool(name="rtop", bufs=3) as rtop_pool,
        ):
            # Split the snt load across both queues: its 128 small
            # descriptors otherwise delay row 0's hankel fan-out by ~8us.
            snt = const_pool.tile([L, K], io_dt)
            nc.sync.dma_start(out=snt[0:L // 2, :], in_=snt_in[0:L // 2, :])
            nc.scalar.dma_start(out=snt[L // 2:L, :], in_=snt_in[L // 2:L, :])
            ident = const_pool.tile([128, 128], f32)
            make_identity(nc, ident[:, :])
            # Result accumulator R[k, m*32 + b], m in (p1, p_mean, p2, dom).
            R = const_pool.tile([K, 128], f32)

            spans = [(0, SPAN), (SPAN, N - SPAN)]   # (n0, n_windows)

            with tc.tile_pool(name="psum", bufs=2, space="PSUM") as psum_pool:
                def emit_y_half(r0):
                    # y rows r0..r0+15: per-metric transposes
                    # R[:, m*32+r0 : m*32+r0+16] -> [16, 128] packed as
                    # tr[b, m*128+k]; y descriptors are full 2KB rows.
                    ps_tr = psum_pool.tile([K, SPAN], f32, tag="psum")
                    for m in range(4):
                        nc.tensor.transpose(
                            ps_tr[0:16, m * K:(m + 1) * K],
                            R[:, m * ROWS + r0:m * ROWS + r0 + 16],
                            ident[:, :])
                    trh = const_pool.tile([16, OUT_COLS], f32,
                                          tag=f"trh{r0}")
                    nc.scalar.copy(trh[:, :], ps_tr[0:16, 0:OUT_COLS])
                    eng = (nc.sync, nc.scalar)[r0 == 0]
                    eng.dma_start(out=y[r0:r0 + 16, :], in_=trh[:, :])

                for b in range(ROWS):
                    # Steady state: whole-row hankel DMAs alternate between
                    # the two HWDGE queues (both stripe over all 16 rings;
                    # interleaving two streams per row was measured slower).
                    # First rows: split across queues to halve the pipeline
                    # fill before the first matmul.
                    h = hank_pool.tile([G, TPAD - G], io_dt, tag="hank")
                    if b < 3:
                        nc.sync.dma_start(out=h[0:G // 2, :],
                                          in_=hankel_ap(b, 0, G // 2))
                        nc.scalar.dma_start(out=h[G // 2:G, :],
                                            in_=hankel_ap(b, G // 2, G // 2))
                    else:
                        eng = (nc.sync, nc.scalar)[b % 2]
                        eng.dma_start(out=h[:, :], in_=hankel_ap(b, 0, G))

                    cand = cand_pool.tile([K, 16], f32)
                    for hi, (n0, nw) in enumerate(spans):
                        ps = psum_pool.tile([K, SPAN], f32, tag="psum")
                        for j in range(0, SPAN, WBLK):
                            c = n0 + j
                            nc.tensor.matmul(
                                ps[:, j:j + WBLK],
                                snt[:, :], h[:, c:c + WBLK],
                                start=True, stop=True)
                        nc.vector.max(out=cand[:, 8 * hi:8 * (hi + 1)],
                                      in_=ps[:, :nw])

                    if b == 16:
                        # First y half-batch (rows 0..15): emitted AFTER
                        # row 16's matmuls so the in-order Tensor queue
                        # doesn't park row-16 matmuls behind transposes
                        # that wait on row-15's scalar finalize.
                        emit_y_half(0)

                    rt = rtop_pool.tile([K, 8], f32)
                    nc.vector.max(out=rt[:, :], in_=cand[:, :])
                    # p1, p_mean, p2, dominance -> R cols b, 32+b, 64+b, 96+b.
                    nc.scalar.copy(R[:, b:b + 1], rt[:, 0:1])
                    pm_scratch = rtop_pool.tile([K, K_TOP], f32, tag="pmscr")
                    nc.scalar.activation(pm_scratch[:, :], rt[:, 0:K_TOP],
                                         mybir.ActivationFunctionType.Copy,
                                         scale=1.0 / K_TOP,
                                         accum_out=R[:, 32 + b:33 + b])
                    nc.scalar.copy(R[:, 64 + b:65 + b], rt[:, 1:2])
                    # dominance = relu(p1 - p2) = Relu(-p2 + bias=p1),
                    # on the scalar engine: stays off the DVE bottleneck.
                    nc.scalar.activation(R[:, 96 + b:97 + b], rt[:, 1:2],
                                         mybir.ActivationFunctionType.Relu,
                                         scale=-1.0, bias=rt[:, 0:1])

                    if b == ROWS - 1:
                        emit_y_half(16)

    _split_excess_waits(nc, mybir)
    return nc


_CACHED = {}


def _get_program():
    if "p" not in _CACHED:
        _CACHED["p"] = build_program()
    return _CACHED["p"]


def _prep_inputs(x, shapelets):
    import ml_dtypes
    x = np.ascontiguousarray(x, dtype=np.float32)
    s = np.asarray(shapelets, dtype=np.float32)
    snt = np.ascontiguousarray((s - s.mean(axis=1, keepdims=True)).T)
    x = np.pad(x, ((0, 0), (0, TPAD - T)))
    x = x.astype(ml_dtypes.bfloat16)
    snt = np.ascontiguousarray(snt).astype(ml_dtypes.bfloat16)
    return x, snt


def run_sharded(x, shapelets, trace=False, **kw):
    from concourse.bass_utils import run_bass_kernel_spmd

    nc = _get_program()
    xp, snt = _prep_inputs(x, shapelets)
    in_maps = [
        {"x": xp[c * ROWS:(c + 1) * ROWS], "snt": snt}
        for c in range(N_CORES)
    ]
    res = run_bass_kernel_spmd(nc, in_maps, list(range(N_CORES)), trace=trace, **kw)
    out = np.concatenate([res.results[c]["y"] for c in range(N_CORES)], axis=0)
    return out, res


def kernel(x, shapelets):
    out, _ = run_sharded(x, shapelets)
    return out
